# revision 2
# baseline (speedup 1.0000x reference)
"""Trainium2 Bass kernel for the B-spline (KAN-style) layer — ARCH-B2.

out[b,f] = sum_k basis_k(x[b,f]) * control_p[k,f] + bias[f], cubic B-spline,
55 uniform intervals on [0,1) (s = 55x integer-knot coordinates).

vs the original baseline (two-level telescope, W=5, 66 chained DVE ops,
device transposes, on-device gtab matmul):
  * W=11 pieces: 60 DVE ops total (5x AB + 55x C).  Every op carries its
    clamp bound in a constant slot (C ops: v = min(t+|t|, C1); AB ops:
    C3-spill bound via in1), so no prescaled width tiles exist and all ops
    read the x tile directly.  The NACC=16 chain heads (all 5 AB + 11 C) are
    single-source ops, so no accumulator init/memset is needed.
  * Host pre-transposes each core's x shard to [feature, batch] and
    transposes the result back: ZERO device transposes.
  * gtab (per-feature coefficients) is computed on the host in float64 from a
    least-squares fit against the exact reference basis (absorbs the
    reference's float32 knot jitter) and DMA'd in directly.
  * The NACC chain accumulators are merged by the TensorEngine (identity
    matmuls, plain fp32 = exact, measured 1e-7) into PSUM — no DVE merge
    tree; ScalarE evacuates.  All non-chain work (init, z tile, evac) is on
    ACT/GPSIMD/PE/DMA and overlaps the DVE stream across loop iterations.
"""

import sys

if "/opt/trn_rl_repo" not in sys.path:
    sys.path.insert(0, "/opt/trn_rl_repo")

import numpy as np

import concourse.bass as bass
import concourse.bacc as bacc
import concourse.tile as tile
from concourse import mybir
from concourse.bass_utils import run_bass_kernel_spmd

BATCH, NF, NK, DG = 4096, 256, 64, 3
NJ = 55
NCORES = 8
BSH, FSH = 1024, 128
F32 = mybir.dt.float32
NACC = 5

PIECES = [0, 11, 22, 33, 44]
PSTARTS = PIECES + [NJ]

# ---------------------------------------------------------------------------
# Host-side spline tables (float64, exact)
# ---------------------------------------------------------------------------

def _knots64():
    dg, nk = DG, NK
    base = np.concatenate([
        np.linspace(-0.002, -0.001, dg),
        np.linspace(0.0, 1.0, nk - 2 * dg - 2),
        np.linspace(1.001, 1.002, dg),
    ])
    dist_lo = base[1] - base[0]
    dist_hi = base[-1] - base[-2]
    left = base[0] - dist_lo * np.arange(dg, 0, -1)
    right = base[-1] + dist_hi * np.arange(1, dg + 1)
    t32 = np.concatenate([left, base, right]).astype(np.float32)
    return t32.astype(np.float64)


def _basis64(x, t):
    xe = x[..., None]
    B = ((t[:-1] <= xe) & (xe < t[1:])).astype(np.float64)
    for k in range(1, DG + 1):
        d1 = t[k:-1] - t[:-k - 1]
        d2 = t[k + 1:] - t[1:-k]
        w1 = np.where(d1 != 0, (xe - t[:-k - 1]) / np.where(d1 != 0, d1, 1.0), 0.0)
        w2 = np.where(d2 != 0, (t[k + 1:] - xe) / np.where(d2 != 0, d2, 1.0), 0.0)
        B = w1 * B[..., :-1] + w2 * B[..., 1:]
    return B  # (..., 64)


def _term_list():
    """Terms evaluated on-device.  AB terms read z = 5x (knot imm2 = 5*kx,
    clamp bound 2 -> width 11 in s units); C terms read x directly (bound in
    C1).  Knots are the reference's float32-rounded positions."""
    kx = _knots64()[2 * DG: 2 * DG + NJ + 1]
    terms = []
    col = 0
    piece_end = {}
    for k, s0 in enumerate(PIECES):
        for m in range(s0, PSTARTS[k + 1]):
            piece_end[m] = PSTARTS[k + 1]
    for k, s0 in enumerate(PIECES):
        e = PSTARTS[k + 1]
        terms.append(dict(kind="ab", imm2=float(kx[s0]),
                          bnd=float(np.float32(2.0 * (kx[e] - kx[s0]))),
                          a_col=col, b_col=col + 1))
        terms.append(dict(kind="c", imm2=float(kx[s0]),
                          s1=float(np.float32(2.0 * (kx[e] - kx[s0]))),
                          col=col + 2))
        col += 3
    for m in range(1, NJ):
        if m in PIECES:
            continue
        e = piece_end[m]
        terms.append(dict(kind="c", imm2=float(kx[m]),
                          s1=float(np.float32(2.0 * (kx[e] - kx[m]))), col=col))
        col += 1
    const_col = col
    col += 1
    return terms, col, const_col


TERMS, NW2, CONST_COL = _term_list()
AB_TERMS = [t for t in TERMS if t["kind"] == "ab"]
NGCOL = NW2 + len(AB_TERMS)


def _phi_matrix(xs):
    """Basis-function values exactly as the device computes them."""
    n = len(xs)
    Phi = np.zeros((n, NW2))
    for tm in TERMS:
        if tm["kind"] == "ab":
            tt = xs - tm["imm2"]
            v = np.minimum(tt + np.abs(tt), tm["bnd"])
            Phi[:, tm["a_col"]] = v
            Phi[:, tm["b_col"]] = v * v
        else:
            tt = xs - tm["imm2"]
            v = np.minimum(tt + np.abs(tt), tm["s1"])
            Phi[:, tm["col"]] = v ** 3
    Phi[:, CONST_COL] = 1.0
    return Phi


_W2_CACHE = {}


def _make_w2():
    """(NK+1, NW2) float64 map [control_p; bias] -> per-feature term
    coefficients, least-squares-fit against the exact reference basis."""
    if "w2" in _W2_CACHE:
        return _W2_CACHE["w2"]
    xs = np.linspace(0.0, 1.0 - 1e-9, 24001)
    Phi = _phi_matrix(xs)
    t = _knots64()
    Y = np.zeros((len(xs), NK + 1))
    Y[:, :NK] = _basis64(xs, t)
    Y[:, NK] = 1.0
    cn = np.linalg.norm(Phi, axis=0)
    cn[cn == 0] = 1.0
    C, *_ = np.linalg.lstsq(Phi / cn, Y, rcond=None)
    w2 = (C / cn[:, None]).T          # (NK+1, NW2)
    _W2_CACHE["w2"] = w2
    return w2


def _selfcheck():
    rng = np.random.default_rng(1)
    cp = rng.standard_normal((NK, 4))
    bias = rng.standard_normal(4)
    t = _knots64()
    xs = np.linspace(0, 1 - 1e-9, 1777)
    ref = _basis64(xs, t) @ cp + bias
    gt = np.concatenate([cp, bias[None, :]], axis=0).T @ _make_w2()  # (4,NW2)
    out = _phi_matrix(xs) @ gt.T
    err = np.abs(out - ref).max() / np.abs(ref).max()
    assert err < 2e-4, f"structural self-check failed: {err}"
    gt32 = gt.astype(np.float32).astype(np.float64)
    out32 = _phi_matrix(xs) @ gt32.T
    err32 = np.abs(out32 - ref).max() / np.abs(ref).max()
    assert err32 < 1e-3, f"fp32 gtab error too large: {err32}"


# ---------------------------------------------------------------------------
# Custom DVE ops (chained)
# ---------------------------------------------------------------------------

def _register_ops():
    from concourse import dve_ops
    from concourse.dve_spec import (
        Spec, Src0, Src1, C0, C1, C2, C3, One, minn, sq, lower, Bin, AluOp,
        _spill_c3_to_src1, _has_src1 as has_src1,
    )
    from concourse.dve_uop import DveOpSpec

    if any(op.name == "BSPL_ABS3_ANT" for op in dve_ops.OPS):
        cb = next(op for op in dve_ops.OPS if op.name == "BSPL_CB_ANT")
        ab = next(op for op in dve_ops.OPS if op.name == "BSPL_ABS3_ANT")
        cs = next(op for op in dve_ops.OPS if op.name == "BSPL_CS_ANT")
        return ab, cb, cs

    # chained C op with bound: out = Src1 + C0 * min(t+|t|, C1)^3, t=Src0-C2
    t1 = Src0 - C2
    v1 = minn(t1 + Bin(AluOp.ABSOLUTE_VALUE, t1, t1), C1)
    body_cb = Src1 + (sq(v1) * v1) * C0

    def ref_cb(in0, in1, s0, s1, imm2):
        tt = in0.astype(np.float32) - np.float32(imm2)
        vv = np.minimum(tt + np.abs(tt), np.asarray(s1, np.float32))
        vv = vv.astype(np.float32)
        return (in1 + vv * vv * vv * np.asarray(s0, np.float32)).astype(np.float32)

    # single-source AB op, bound via C3 (in1): out = v*(C0+C1*v), v=min(t+|t|,C3)
    t2 = Src0 - C2
    v2 = minn(t2 + Bin(AluOp.ABSOLUTE_VALUE, t2, t2), C3)
    body_ab = v2 * (C0 + C1 * v2)

    def ref_ab(in0, in1, s0, s1, imm2):
        tt = in0.astype(np.float32) - np.float32(imm2)
        vv = np.minimum(tt + np.abs(tt), in1.astype(np.float32)).astype(np.float32)
        return (vv * (np.asarray(s0, np.float32)
                      + np.asarray(s1, np.float32) * vv)).astype(np.float32)

    def _mk(name, spec):
        shas = {}
        for ver in ("v3", "v4"):
            probe = DveOpSpec(name=name, opcode=0,
                              uops=lower(spec, ver=ver), rd1_en=has_src1(spec))
            shas[ver] = probe.sha(ver)
        op = dve_ops.DveOp(name, spec, subdim=False, uops_sha=shas)
        dve_ops.OPS.append(op)
        dve_ops.CUSTOM_DVE_SPECS[name] = spec
        row = dve_ops._CUSTOM_DVE_ROW_BASE + len(dve_ops.OPS) - 1
        assert row < 0x20
        dve_ops._SUB_OPCODE_FOR_NAME[name] = row
        return op

    # single-source C op (chain head, no accumulate): out = C0*min(t+|t|,C1)^3
    t3 = Src0 - C2
    v3 = minn(t3 + Bin(AluOp.ABSOLUTE_VALUE, t3, t3), C1)
    body_cs = (sq(v3) * v3) * C0

    def ref_cs(in0, in1, s0, s1, imm2):
        tt = in0.astype(np.float32) - np.float32(imm2)
        vv = np.minimum(tt + np.abs(tt), np.asarray(s1, np.float32))
        vv = vv.astype(np.float32)
        return (vv * vv * vv * np.asarray(s0, np.float32)).astype(np.float32)

    cb = _mk("BSPL_CB_ANT", Spec(body=body_cb, reference=ref_cb))
    ab = _mk("BSPL_ABS3_ANT",
             Spec(body=_spill_c3_to_src1(body_ab), reference=ref_ab))
    cs = _mk("BSPL_CS_ANT", Spec(body=body_cs, reference=ref_cs))
    return ab, cb, cs


# ---------------------------------------------------------------------------
# Bass kernel
# ---------------------------------------------------------------------------

_CACHE = {}


def _build_module(body_reps=1, nterms=None):
    key = ("e", body_reps, nterms)
    if key in _CACHE:
        return _CACHE[key]
    op_ab, op_cb, op_cs = _register_ops()
    from concourse import masks
    import contextlib

    terms = TERMS if nterms is None else TERMS[:nterms]
    # chain heads are single-source: all AB terms (C3-bound) + C fills
    c_terms = [t for t in terms if t["kind"] == "c"]
    ab_terms = [t for t in terms if t["kind"] == "ab"]
    nch = max(0, NACC - len(ab_terms))
    heads = ab_terms + c_terms[:nch]
    rest = c_terms[nch:]
    terms = heads + rest
    assert len(heads) == NACC

    nc = bacc.Bacc("TRN2", target_bir_lowering=False, debug=False,
                   num_devices=NCORES)
    x_in = nc.dram_tensor("x", [FSH, BSH], F32, kind="ExternalInput").ap()
    g_in = nc.dram_tensor("gtab", [FSH, NGCOL], F32, kind="ExternalInput").ap()
    y_out = nc.dram_tensor("y", [FSH, BSH], F32, kind="ExternalOutput").ap()

    with tile.TileContext(nc) as tc:
        with contextlib.ExitStack() as _st:
            const_pool = _st.enter_context(tc.tile_pool(name="const", bufs=1))
            x_pool = _st.enter_context(tc.tile_pool(name="x", bufs=2))
            z_pool = _st.enter_context(tc.tile_pool(name="z", bufs=2))
            a_pool = _st.enter_context(tc.tile_pool(name="a", bufs=2))
            out_pool = _st.enter_context(tc.tile_pool(name="out", bufs=2))
            ps_pool = _st.enter_context(
                tc.tile_pool(name="ps", bufs=2, space="PSUM"))

            identf = const_pool.tile([128, 128], F32)
            masks.make_identity(nc, identf[:])
            gtab = const_pool.tile([FSH, NGCOL], F32)
            nc.sync.dma_start(gtab[:], g_in[:])

            if body_reps > 1:
                _st.enter_context(tc.For_i(0, body_reps, 1))

            xt = x_pool.tile([FSH, BSH], F32, name="xt", tag="xt")
            nc.sync.dma_start(xt[:], x_in[:])

            nab = 0
            accs = [a_pool.tile([FSH, BSH], F32, name=f"acc{i}", tag=f"acc{i}")
                    for i in range(NACC)]
            for i, tm in enumerate(terms):
                acc = accs[i % NACC]
                if tm["kind"] == "ab":     # head: single-source, C3 bound
                    nc.vector._custom_dve(
                        op_ab, out=acc[:], in0=xt[:],
                        in1=gtab[:, NW2 + nab:NW2 + nab + 1],
                        s0=gtab[:, tm["a_col"]:tm["a_col"] + 1],
                        s1=gtab[:, tm["b_col"]:tm["b_col"] + 1],
                        imm2=tm["imm2"])
                    nab += 1
                elif i < NACC:      # chain head: single-source, no accumulate
                    nc.vector._custom_dve(
                        op_cs, out=acc[:], in0=xt[:],
                        s0=gtab[:, tm["col"]:tm["col"] + 1],
                        s1=tm["s1"], imm2=tm["imm2"])
                else:
                    nc.vector._custom_dve(
                        op_cb, out=acc[:], in0=xt[:], in1=acc[:],
                        s0=gtab[:, tm["col"]:tm["col"] + 1],
                        s1=tm["s1"], imm2=tm["imm2"])

            # merge the chain accumulators on the TensorEngine (fp32, exact)
            acc_ps = ps_pool.tile([FSH, BSH], F32, name="accps", tag="accps")
            for i, t in enumerate(accs):
                for h in range(2):
                    sl = slice(h * 512, (h + 1) * 512)
                    nc.tensor.matmul(acc_ps[:, sl], identf[:], t[:, sl],
                                     start=(i == 0), stop=(i == NACC - 1))
            # evac with the constant+bias column folded in; the only per-rep
            # ACT op, so it overlaps the next rep's DVE stream
            yt = out_pool.tile([FSH, BSH], F32, name="yt", tag="yt")
            nc.scalar.activation(yt[:], acc_ps[:],
                                 mybir.ActivationFunctionType.Identity,
                                 bias=gtab[:, CONST_COL:CONST_COL + 1],
                                 scale=1.0)
            nc.sync.dma_start(y_out[:], yt[:])

    nc.compile()
    _CACHE[key] = nc
    return nc


# ---------------------------------------------------------------------------
# Public entry point
# ---------------------------------------------------------------------------

_CHECKED = False


def _make_in_maps(x, control_p, bias):
    global _CHECKED
    if not _CHECKED:
        _selfcheck()
        _CHECKED = True
    x = np.ascontiguousarray(x, dtype=np.float32)
    control_p = np.ascontiguousarray(control_p, dtype=np.float32)
    bias = np.ascontiguousarray(bias, dtype=np.float32)
    assert x.shape == (BATCH, NF) and control_p.shape == (NK, NF)
    w2 = _make_w2()     # (NK+1, NW2) float64
    in_maps, slots = [], []
    for c in range(NCORES):
        fh, bq = c // 4, c % 4
        fsl = slice(fh * FSH, (fh + 1) * FSH)
        bsl = slice(bq * BSH, (bq + 1) * BSH)
        cpb = np.concatenate([control_p[:, fsl], bias[None, fsl]],
                             axis=0).astype(np.float64)
        gtab = (cpb.T @ w2).astype(np.float32)
        bnds = np.zeros((FSH, len(AB_TERMS)), dtype=np.float32)
        for j, tm in enumerate(AB_TERMS):
            bnds[:, j] = np.float32(tm["bnd"])
        gtab = np.ascontiguousarray(np.concatenate([gtab, bnds], axis=1))
        in_maps.append({
            "x": np.ascontiguousarray(x[bsl, fsl].T),   # [FSH, BSH]
            "gtab": gtab,
        })
        slots.append((bsl, fsl))
    return in_maps, slots


def kernel(x, control_p, bias):
    nc = _build_module()
    in_maps, slots = _make_in_maps(x, control_p, bias)
    res = run_bass_kernel_spmd(nc, in_maps, list(range(NCORES)))

    out = np.empty((BATCH, NF), dtype=np.float32)
    for c, (bsl, fsl) in enumerate(slots):
        out[bsl, fsl] = res.results[c]["y"].T
    return out



# revision 8
# speedup vs baseline: 1.0195x; 1.0195x over previous
"""Trainium2 Bass kernel for the B-spline (KAN-style) layer — ARCH-B2.

out[b,f] = sum_k basis_k(x[b,f]) * control_p[k,f] + bias[f], cubic B-spline,
55 uniform intervals on [0,1) (s = 55x integer-knot coordinates).

vs the original baseline (two-level telescope, W=5, 66 chained DVE ops,
device transposes, on-device gtab matmul):
  * W=11 pieces: 60 DVE ops total (5x AB + 55x C).  Every op carries its
    clamp bound in a constant slot (C ops: v = min(t+|t|, C1); AB ops:
    C3-spill bound via in1), so no prescaled width tiles exist and all ops
    read the x tile directly.  The NACC=16 chain heads (all 5 AB + 11 C) are
    single-source ops, so no accumulator init/memset is needed.
  * Host pre-transposes each core's x shard to [feature, batch] and
    transposes the result back: ZERO device transposes.
  * gtab (per-feature coefficients) is computed on the host in float64 from a
    least-squares fit against the exact reference basis (absorbs the
    reference's float32 knot jitter) and DMA'd in directly.
  * The NACC chain accumulators are merged by the TensorEngine (identity
    matmuls, plain fp32 = exact, measured 1e-7) into PSUM — no DVE merge
    tree; ScalarE evacuates.  All non-chain work (init, z tile, evac) is on
    ACT/GPSIMD/PE/DMA and overlaps the DVE stream across loop iterations.
"""

import sys

if "/opt/trn_rl_repo" not in sys.path:
    sys.path.insert(0, "/opt/trn_rl_repo")

import numpy as np

import concourse.bass as bass
import concourse.bacc as bacc
import concourse.tile as tile
from concourse import mybir
from concourse.bass_utils import run_bass_kernel_spmd

BATCH, NF, NK, DG = 4096, 256, 64, 3
NJ = 55
NCORES = 8
BSH, FSH = 1024, 128
F32 = mybir.dt.float32
NACC = 5

PIECES = [0, 11, 22, 33, 44]
PSTARTS = PIECES + [NJ]

# ---------------------------------------------------------------------------
# Host-side spline tables (float64, exact)
# ---------------------------------------------------------------------------

def _knots64():
    dg, nk = DG, NK
    base = np.concatenate([
        np.linspace(-0.002, -0.001, dg),
        np.linspace(0.0, 1.0, nk - 2 * dg - 2),
        np.linspace(1.001, 1.002, dg),
    ])
    dist_lo = base[1] - base[0]
    dist_hi = base[-1] - base[-2]
    left = base[0] - dist_lo * np.arange(dg, 0, -1)
    right = base[-1] + dist_hi * np.arange(1, dg + 1)
    t32 = np.concatenate([left, base, right]).astype(np.float32)
    return t32.astype(np.float64)


def _basis64(x, t):
    xe = x[..., None]
    B = ((t[:-1] <= xe) & (xe < t[1:])).astype(np.float64)
    for k in range(1, DG + 1):
        d1 = t[k:-1] - t[:-k - 1]
        d2 = t[k + 1:] - t[1:-k]
        w1 = np.where(d1 != 0, (xe - t[:-k - 1]) / np.where(d1 != 0, d1, 1.0), 0.0)
        w2 = np.where(d2 != 0, (t[k + 1:] - xe) / np.where(d2 != 0, d2, 1.0), 0.0)
        B = w1 * B[..., :-1] + w2 * B[..., 1:]
    return B  # (..., 64)


def _term_list():
    """Terms evaluated on-device.  AB terms read z = 5x (knot imm2 = 5*kx,
    clamp bound 2 -> width 11 in s units); C terms read x directly (bound in
    C1).  Knots are the reference's float32-rounded positions."""
    kx = _knots64()[2 * DG: 2 * DG + NJ + 1]
    terms = []
    col = 0
    piece_end = {}
    for k, s0 in enumerate(PIECES):
        for m in range(s0, PSTARTS[k + 1]):
            piece_end[m] = PSTARTS[k + 1]
    for k, s0 in enumerate(PIECES):
        e = PSTARTS[k + 1]
        terms.append(dict(kind="ab", imm2=float(kx[s0]),
                          bnd=float(np.float32(2.0 * (kx[e] - kx[s0]))),
                          a_col=col, b_col=col + 1))
        terms.append(dict(kind="c", imm2=float(kx[s0]),
                          s1=float(np.float32(2.0 * (kx[e] - kx[s0]))),
                          col=col + 2))
        col += 3
    for m in range(1, NJ):
        if m in PIECES:
            continue
        e = piece_end[m]
        terms.append(dict(kind="c", imm2=float(kx[m]),
                          s1=float(np.float32(2.0 * (kx[e] - kx[m]))), col=col))
        col += 1
    const_col = col
    col += 1
    return terms, col, const_col


TERMS, NW2, CONST_COL = _term_list()
AB_TERMS = [t for t in TERMS if t["kind"] == "ab"]
NGCOL = NW2 + len(AB_TERMS)


def _phi_matrix(xs):
    """Basis-function values exactly as the device computes them."""
    n = len(xs)
    Phi = np.zeros((n, NW2))
    for tm in TERMS:
        if tm["kind"] == "ab":
            tt = xs - tm["imm2"]
            v = np.minimum(tt + np.abs(tt), tm["bnd"])
            Phi[:, tm["a_col"]] = v
            Phi[:, tm["b_col"]] = v * v
        else:
            tt = xs - tm["imm2"]
            v = np.minimum(tt + np.abs(tt), tm["s1"])
            Phi[:, tm["col"]] = v ** 3
    Phi[:, CONST_COL] = 1.0
    return Phi


_W2_CACHE = {}


def _make_w2():
    """(NK+1, NW2) float64 map [control_p; bias] -> per-feature term
    coefficients, least-squares-fit against the exact reference basis."""
    if "w2" in _W2_CACHE:
        return _W2_CACHE["w2"]
    xs = np.linspace(0.0, 1.0 - 1e-9, 24001)
    Phi = _phi_matrix(xs)
    t = _knots64()
    Y = np.zeros((len(xs), NK + 1))
    Y[:, :NK] = _basis64(xs, t)
    Y[:, NK] = 1.0
    cn = np.linalg.norm(Phi, axis=0)
    cn[cn == 0] = 1.0
    C, *_ = np.linalg.lstsq(Phi / cn, Y, rcond=None)
    w2 = (C / cn[:, None]).T          # (NK+1, NW2)
    _W2_CACHE["w2"] = w2
    return w2


def _selfcheck():
    rng = np.random.default_rng(1)
    cp = rng.standard_normal((NK, 4))
    bias = rng.standard_normal(4)
    t = _knots64()
    xs = np.linspace(0, 1 - 1e-9, 1777)
    ref = _basis64(xs, t) @ cp + bias
    gt = np.concatenate([cp, bias[None, :]], axis=0).T @ _make_w2()  # (4,NW2)
    out = _phi_matrix(xs) @ gt.T
    err = np.abs(out - ref).max() / np.abs(ref).max()
    assert err < 2e-4, f"structural self-check failed: {err}"
    gt32 = gt.astype(np.float32).astype(np.float64)
    out32 = _phi_matrix(xs) @ gt32.T
    err32 = np.abs(out32 - ref).max() / np.abs(ref).max()
    assert err32 < 1e-3, f"fp32 gtab error too large: {err32}"


# ---------------------------------------------------------------------------
# Custom DVE ops (chained)
# ---------------------------------------------------------------------------

def _register_ops():
    from concourse import dve_ops
    from concourse.dve_spec import (
        Spec, Src0, Src1, C0, C1, C2, C3, One, minn, sq, lower, Bin, AluOp,
        _spill_c3_to_src1, _has_src1 as has_src1,
    )
    from concourse.dve_uop import DveOpSpec

    if any(op.name == "BSPL_ABS3_ANT" for op in dve_ops.OPS):
        cb = next(op for op in dve_ops.OPS if op.name == "BSPL_CB_ANT")
        ab = next(op for op in dve_ops.OPS if op.name == "BSPL_ABS3_ANT")
        cs = next(op for op in dve_ops.OPS if op.name == "BSPL_CS_ANT")
        return ab, cb, cs

    # chained C op with bound: out = Src1 + C0 * min(t+|t|, C1)^3, t=Src0-C2
    t1 = Src0 - C2
    v1 = minn(t1 + Bin(AluOp.ABSOLUTE_VALUE, t1, t1), C1)
    body_cb = Src1 + (sq(v1) * v1) * C0

    def ref_cb(in0, in1, s0, s1, imm2):
        tt = in0.astype(np.float32) - np.float32(imm2)
        vv = np.minimum(tt + np.abs(tt), np.asarray(s1, np.float32))
        vv = vv.astype(np.float32)
        return (in1 + vv * vv * vv * np.asarray(s0, np.float32)).astype(np.float32)

    # single-source AB op, bound via C3 (in1): out = v*(C0+C1*v), v=min(t+|t|,C3)
    t2 = Src0 - C2
    v2 = minn(t2 + Bin(AluOp.ABSOLUTE_VALUE, t2, t2), C3)
    body_ab = v2 * (C0 + C1 * v2)

    def ref_ab(in0, in1, s0, s1, imm2):
        tt = in0.astype(np.float32) - np.float32(imm2)
        vv = np.minimum(tt + np.abs(tt), in1.astype(np.float32)).astype(np.float32)
        return (vv * (np.asarray(s0, np.float32)
                      + np.asarray(s1, np.float32) * vv)).astype(np.float32)

    def _mk(name, spec):
        shas = {}
        for ver in ("v3", "v4"):
            probe = DveOpSpec(name=name, opcode=0,
                              uops=lower(spec, ver=ver), rd1_en=has_src1(spec))
            shas[ver] = probe.sha(ver)
        op = dve_ops.DveOp(name, spec, subdim=False, uops_sha=shas)
        dve_ops.OPS.append(op)
        dve_ops.CUSTOM_DVE_SPECS[name] = spec
        row = dve_ops._CUSTOM_DVE_ROW_BASE + len(dve_ops.OPS) - 1
        assert row < 0x20
        dve_ops._SUB_OPCODE_FOR_NAME[name] = row
        return op

    # single-source C op (chain head, no accumulate): out = C0*min(t+|t|,C1)^3
    t3 = Src0 - C2
    v3 = minn(t3 + Bin(AluOp.ABSOLUTE_VALUE, t3, t3), C1)
    body_cs = (sq(v3) * v3) * C0

    def ref_cs(in0, in1, s0, s1, imm2):
        tt = in0.astype(np.float32) - np.float32(imm2)
        vv = np.minimum(tt + np.abs(tt), np.asarray(s1, np.float32))
        vv = vv.astype(np.float32)
        return (vv * vv * vv * np.asarray(s0, np.float32)).astype(np.float32)

    cb = _mk("BSPL_CB_ANT", Spec(body=body_cb, reference=ref_cb))
    ab = _mk("BSPL_ABS3_ANT",
             Spec(body=_spill_c3_to_src1(body_ab), reference=ref_ab))
    cs = _mk("BSPL_CS_ANT", Spec(body=body_cs, reference=ref_cs))
    return ab, cb, cs


# ---------------------------------------------------------------------------
# Bass kernel
# ---------------------------------------------------------------------------

_CACHE = {}


def _build_module(body_reps=1, nterms=None):
    key = ("e", body_reps, nterms)
    if key in _CACHE:
        return _CACHE[key]
    op_ab, op_cb, op_cs = _register_ops()
    from concourse import masks
    import contextlib

    terms = TERMS if nterms is None else TERMS[:nterms]
    # Piece-aligned accumulator chains: acc p owns piece p's AB head (the
    # single-source chain head) plus its C terms.  Emission alternates
    # between the two lowest-index still-open accs so acc 0 closes after
    # ~2x its term count; each acc's PE merge is emitted right after its
    # chain closes and overlaps the remaining DVE stream — only the last
    # acc's merge is tail-latency.
    import bisect
    ab_terms = [t for t in terms if t["kind"] == "ab"]
    assert len(ab_terms) == NACC
    kx_pieces = [float(_knots64()[2 * DG + s]) for s in PIECES]
    acc_terms = [[] for _ in range(NACC)]
    for t in terms:
        p = bisect.bisect_right(kx_pieces, t["imm2"] + 1e-12) - 1
        acc_terms[p].append(t)
    for p in range(NACC):
        # AB head must come first within its acc
        acc_terms[p].sort(key=lambda t: 0 if t["kind"] == "ab" else 1)
        assert acc_terms[p][0]["kind"] == "ab"

    def _emit_order():
        """Acc 4 (the PSUM-resident chain) takes every 5th slot so it closes
        on the final op; accs 0,1 alternate through the first half, 2,3
        through the second.  All same-acc spacings are >= 2 ops, so chained
        Src1 reads never stall on the previous write's ack."""
        n = sum(len(a) for a in acc_terms)
        rem = [len(a) for a in acc_terms]
        order = []
        phase = [0, 1]
        for pos in range(n):
            if pos % 5 == 4 and rem[4] > 0:
                p = 4
            else:
                act = [q for q in phase if rem[q] > 0]
                if not act:
                    phase = [2, 3]
                    act = [q for q in phase if rem[q] > 0]
                if not act:
                    act = [q for q in range(NACC) if rem[q] > 0]
                p = act[0]
                if order and order[-1] == p and len(act) > 1:
                    p = act[1]
            order.append(p)
            rem[p] -= 1
        return order

    nc = bacc.Bacc("TRN2", target_bir_lowering=False, debug=False,
                   num_devices=NCORES)
    x_in = nc.dram_tensor("x", [FSH, BSH], F32, kind="ExternalInput").ap()
    g_in = nc.dram_tensor("gtab", [FSH, NGCOL], F32, kind="ExternalInput").ap()
    y_out = nc.dram_tensor("y", [FSH, BSH], F32, kind="ExternalOutput").ap()

    with tile.TileContext(nc) as tc:
        with contextlib.ExitStack() as _st:
            const_pool = _st.enter_context(tc.tile_pool(name="const", bufs=1))
            x_pool = _st.enter_context(tc.tile_pool(name="x", bufs=2))
            z_pool = _st.enter_context(tc.tile_pool(name="z", bufs=2))
            a_pool = _st.enter_context(tc.tile_pool(name="a", bufs=2))
            out_pool = _st.enter_context(tc.tile_pool(name="out", bufs=2))
            ps_pool = _st.enter_context(
                tc.tile_pool(name="ps", bufs=2, space="PSUM"))

            identf = const_pool.tile([128, 128], F32)
            masks.make_identity(nc, identf[:])
            gtab = const_pool.tile([FSH, NGCOL], F32)
            nc.sync.dma_start(gtab[:], g_in[:])

            if body_reps > 1:
                _st.enter_context(tc.For_i(0, body_reps, 1))

            xt = x_pool.tile([FSH, BSH], F32, name="xt", tag="xt")
            nc.sync.dma_start(xt[:], x_in[:])

            # accs 0..3 get PE-merged into acc_ps as their chains close; acc 4
            # closes on the final DVE op and is read directly by the evac STT
            # (PSUM allows only one non-scalar STT input), so it needs no
            # merge at all.
            accs = [a_pool.tile([FSH, BSH], F32, name=f"acc{i}", tag=f"acc{i}")
                    for i in range(NACC)]
            acc_ps = ps_pool.tile([FSH, BSH], F32, name="accps", tag="accps")
            acc_e = accs[NACC - 1]

            ab_idx = {id(t): j for j, t in enumerate(ab_terms)}
            emitted = [0] * NACC
            merged = 0
            for p in _emit_order():
                tm = acc_terms[p][emitted[p]]
                acc = accs[p]
                if tm["kind"] == "ab":     # chain head: single-source, C3 bound
                    nab = ab_idx[id(tm)]
                    nc.vector._custom_dve(
                        op_ab, out=acc[:], in0=xt[:],
                        in1=gtab[:, NW2 + nab:NW2 + nab + 1],
                        s0=gtab[:, tm["a_col"]:tm["a_col"] + 1],
                        s1=gtab[:, tm["b_col"]:tm["b_col"] + 1],
                        imm2=tm["imm2"])
                else:
                    nc.vector._custom_dve(
                        op_cb, out=acc[:], in0=xt[:], in1=acc[:],
                        s0=gtab[:, tm["col"]:tm["col"] + 1],
                        s1=tm["s1"], imm2=tm["imm2"])
                emitted[p] += 1
                if emitted[p] == len(acc_terms[p]) and p < NACC - 1:
                    # SBUF chain p just closed: merge it into PSUM now so the
                    # matmuls overlap the remaining DVE stream
                    for h in range(2):
                        sl = slice(h * 512, (h + 1) * 512)
                        nc.tensor.matmul(acc_ps[:, sl], identf[:],
                                         acc[:, sl],
                                         start=(merged == 0),
                                         stop=(merged == NACC - 2))
                    merged += 1
            # evac on DVE: yt = (acc_ps + bias) + acc_e; replaces the ACT
            # identity evac and the 5th chain's merge
            yt = out_pool.tile([FSH, BSH], F32, name="yt", tag="yt")
            nc.vector.scalar_tensor_tensor(
                yt[:], acc_ps[:], gtab[:, CONST_COL:CONST_COL + 1], acc_e[:],
                op0=mybir.AluOpType.add, op1=mybir.AluOpType.add)
            nc.sync.dma_start(y_out[:], yt[:])

    nc.compile()
    _CACHE[key] = nc
    return nc


# ---------------------------------------------------------------------------
# Public entry point
# ---------------------------------------------------------------------------

_CHECKED = False


def _make_in_maps(x, control_p, bias):
    global _CHECKED
    if not _CHECKED:
        _selfcheck()
        _CHECKED = True
    x = np.ascontiguousarray(x, dtype=np.float32)
    control_p = np.ascontiguousarray(control_p, dtype=np.float32)
    bias = np.ascontiguousarray(bias, dtype=np.float32)
    assert x.shape == (BATCH, NF) and control_p.shape == (NK, NF)
    w2 = _make_w2()     # (NK+1, NW2) float64
    in_maps, slots = [], []
    for c in range(NCORES):
        fh, bq = c // 4, c % 4
        fsl = slice(fh * FSH, (fh + 1) * FSH)
        bsl = slice(bq * BSH, (bq + 1) * BSH)
        cpb = np.concatenate([control_p[:, fsl], bias[None, fsl]],
                             axis=0).astype(np.float64)
        gtab = (cpb.T @ w2).astype(np.float32)
        bnds = np.zeros((FSH, len(AB_TERMS)), dtype=np.float32)
        for j, tm in enumerate(AB_TERMS):
            bnds[:, j] = np.float32(tm["bnd"])
        gtab = np.ascontiguousarray(np.concatenate([gtab, bnds], axis=1))
        in_maps.append({
            "x": np.ascontiguousarray(x[bsl, fsl].T),   # [FSH, BSH]
            "gtab": gtab,
        })
        slots.append((bsl, fsl))
    return in_maps, slots


def kernel(x, control_p, bias):
    nc = _build_module()
    in_maps, slots = _make_in_maps(x, control_p, bias)
    res = run_bass_kernel_spmd(nc, in_maps, list(range(NCORES)))

    out = np.empty((BATCH, NF), dtype=np.float32)
    for c, (bsl, fsl) in enumerate(slots):
        out[bsl, fsl] = res.results[c]["y"].T
    return out



# revision 10
# speedup vs baseline: 1.0885x; 1.0677x over previous
"""Trainium2 Bass kernel for the B-spline (KAN-style) layer — ARCH-B2.

out[b,f] = sum_k basis_k(x[b,f]) * control_p[k,f] + bias[f], cubic B-spline,
55 uniform intervals on [0,1) (s = 55x integer-knot coordinates).

vs the original baseline (two-level telescope, W=5, 66 chained DVE ops,
device transposes, on-device gtab matmul):
  * W=11 pieces: 60 DVE ops total (5x AB + 55x C).  Every op carries its
    clamp bound in a constant slot (C ops: v = min(t+|t|, C1); AB ops:
    C3-spill bound via in1), so no prescaled width tiles exist and all ops
    read the x tile directly.  The NACC=16 chain heads (all 5 AB + 11 C) are
    single-source ops, so no accumulator init/memset is needed.
  * Host pre-transposes each core's x shard to [feature, batch] and
    transposes the result back: ZERO device transposes.
  * gtab (per-feature coefficients) is computed on the host in float64 from a
    least-squares fit against the exact reference basis (absorbs the
    reference's float32 knot jitter) and DMA'd in directly.
  * The NACC chain accumulators are merged by the TensorEngine (identity
    matmuls, plain fp32 = exact, measured 1e-7) into PSUM — no DVE merge
    tree; ScalarE evacuates.  All non-chain work (init, z tile, evac) is on
    ACT/GPSIMD/PE/DMA and overlaps the DVE stream across loop iterations.
"""

import sys

if "/opt/trn_rl_repo" not in sys.path:
    sys.path.insert(0, "/opt/trn_rl_repo")

import numpy as np

import concourse.bass as bass
import concourse.bacc as bacc
import concourse.tile as tile
from concourse import mybir
from concourse.bass_utils import run_bass_kernel_spmd

BATCH, NF, NK, DG = 4096, 256, 64, 3
NJ = 55
NCORES = 8
BSH, FSH = 1024, 128
F32 = mybir.dt.float32
NACC = 5

PIECES = [0, 11, 22, 33, 44]
PSTARTS = PIECES + [NJ]

# ---------------------------------------------------------------------------
# Host-side spline tables (float64, exact)
# ---------------------------------------------------------------------------

def _knots64():
    dg, nk = DG, NK
    base = np.concatenate([
        np.linspace(-0.002, -0.001, dg),
        np.linspace(0.0, 1.0, nk - 2 * dg - 2),
        np.linspace(1.001, 1.002, dg),
    ])
    dist_lo = base[1] - base[0]
    dist_hi = base[-1] - base[-2]
    left = base[0] - dist_lo * np.arange(dg, 0, -1)
    right = base[-1] + dist_hi * np.arange(1, dg + 1)
    t32 = np.concatenate([left, base, right]).astype(np.float32)
    return t32.astype(np.float64)


def _basis64(x, t):
    xe = x[..., None]
    B = ((t[:-1] <= xe) & (xe < t[1:])).astype(np.float64)
    for k in range(1, DG + 1):
        d1 = t[k:-1] - t[:-k - 1]
        d2 = t[k + 1:] - t[1:-k]
        w1 = np.where(d1 != 0, (xe - t[:-k - 1]) / np.where(d1 != 0, d1, 1.0), 0.0)
        w2 = np.where(d2 != 0, (t[k + 1:] - xe) / np.where(d2 != 0, d2, 1.0), 0.0)
        B = w1 * B[..., :-1] + w2 * B[..., 1:]
    return B  # (..., 64)


def _term_list():
    """Terms evaluated on-device.  AB terms read z = 5x (knot imm2 = 5*kx,
    clamp bound 2 -> width 11 in s units); C terms read x directly (bound in
    C1).  Knots are the reference's float32-rounded positions."""
    kx = _knots64()[2 * DG: 2 * DG + NJ + 1]
    terms = []
    col = 0
    piece_end = {}
    for k, s0 in enumerate(PIECES):
        for m in range(s0, PSTARTS[k + 1]):
            piece_end[m] = PSTARTS[k + 1]
    for k, s0 in enumerate(PIECES):
        e = PSTARTS[k + 1]
        terms.append(dict(kind="ab", imm2=float(kx[s0]),
                          bnd=float(np.float32(2.0 * (kx[e] - kx[s0]))),
                          a_col=col, b_col=col + 1))
        terms.append(dict(kind="c", imm2=float(kx[s0]),
                          s1=float(np.float32(2.0 * (kx[e] - kx[s0]))),
                          col=col + 2))
        col += 3
    for m in range(1, NJ):
        if m in PIECES:
            continue
        e = piece_end[m]
        terms.append(dict(kind="c", imm2=float(kx[m]),
                          s1=float(np.float32(2.0 * (kx[e] - kx[m]))), col=col))
        col += 1
    const_col = col
    col += 1
    return terms, col, const_col


TERMS, NW2, CONST_COL = _term_list()
AB_TERMS = [t for t in TERMS if t["kind"] == "ab"]
NGCOL = NW2 + len(AB_TERMS)


def _phi_matrix(xs):
    """Basis-function values exactly as the device computes them."""
    n = len(xs)
    Phi = np.zeros((n, NW2))
    for tm in TERMS:
        if tm["kind"] == "ab":
            tt = xs - tm["imm2"]
            v = np.minimum(tt + np.abs(tt), tm["bnd"])
            Phi[:, tm["a_col"]] = v
            Phi[:, tm["b_col"]] = v * v
        else:
            tt = xs - tm["imm2"]
            v = np.minimum(tt + np.abs(tt), tm["s1"])
            Phi[:, tm["col"]] = v ** 3
    Phi[:, CONST_COL] = 1.0
    return Phi


_W2_CACHE = {}


def _make_w2():
    """(NK+1, NW2) float64 map [control_p; bias] -> per-feature term
    coefficients, least-squares-fit against the exact reference basis."""
    if "w2" in _W2_CACHE:
        return _W2_CACHE["w2"]
    xs = np.linspace(0.0, 1.0 - 1e-9, 24001)
    Phi = _phi_matrix(xs)
    t = _knots64()
    Y = np.zeros((len(xs), NK + 1))
    Y[:, :NK] = _basis64(xs, t)
    Y[:, NK] = 1.0
    cn = np.linalg.norm(Phi, axis=0)
    cn[cn == 0] = 1.0
    C, *_ = np.linalg.lstsq(Phi / cn, Y, rcond=None)
    w2 = (C / cn[:, None]).T          # (NK+1, NW2)
    _W2_CACHE["w2"] = w2
    return w2


def _selfcheck():
    rng = np.random.default_rng(1)
    cp = rng.standard_normal((NK, 4))
    bias = rng.standard_normal(4)
    t = _knots64()
    xs = np.linspace(0, 1 - 1e-9, 1777)
    ref = _basis64(xs, t) @ cp + bias
    gt = np.concatenate([cp, bias[None, :]], axis=0).T @ _make_w2()  # (4,NW2)
    out = _phi_matrix(xs) @ gt.T
    err = np.abs(out - ref).max() / np.abs(ref).max()
    assert err < 2e-4, f"structural self-check failed: {err}"
    gt32 = gt.astype(np.float32).astype(np.float64)
    out32 = _phi_matrix(xs) @ gt32.T
    err32 = np.abs(out32 - ref).max() / np.abs(ref).max()
    assert err32 < 1e-3, f"fp32 gtab error too large: {err32}"


# ---------------------------------------------------------------------------
# Custom DVE ops (chained)
# ---------------------------------------------------------------------------

def _register_ops():
    from concourse import dve_ops
    from concourse.dve_spec import (
        Spec, Src0, Src1, C0, C1, C2, C3, One, minn, sq, lower, Bin, AluOp,
        _spill_c3_to_src1, _has_src1 as has_src1,
    )
    from concourse.dve_uop import DveOpSpec

    if any(op.name == "BSPL_ABS3_ANT" for op in dve_ops.OPS):
        cb = next(op for op in dve_ops.OPS if op.name == "BSPL_CB_ANT")
        ab = next(op for op in dve_ops.OPS if op.name == "BSPL_ABS3_ANT")
        cs = next(op for op in dve_ops.OPS if op.name == "BSPL_CS_ANT")
        return ab, cb, cs

    # chained C op with bound: out = Src1 + C0 * min(t+|t|, C1)^3, t=Src0-C2
    t1 = Src0 - C2
    v1 = minn(t1 + Bin(AluOp.ABSOLUTE_VALUE, t1, t1), C1)
    body_cb = Src1 + (sq(v1) * v1) * C0

    def ref_cb(in0, in1, s0, s1, imm2):
        tt = in0.astype(np.float32) - np.float32(imm2)
        vv = np.minimum(tt + np.abs(tt), np.asarray(s1, np.float32))
        vv = vv.astype(np.float32)
        return (in1 + vv * vv * vv * np.asarray(s0, np.float32)).astype(np.float32)

    # single-source AB op, bound via C3 (in1): out = v*(C0+C1*v), v=min(t+|t|,C3)
    t2 = Src0 - C2
    v2 = minn(t2 + Bin(AluOp.ABSOLUTE_VALUE, t2, t2), C3)
    body_ab = v2 * (C0 + C1 * v2)

    def ref_ab(in0, in1, s0, s1, imm2):
        tt = in0.astype(np.float32) - np.float32(imm2)
        vv = np.minimum(tt + np.abs(tt), in1.astype(np.float32)).astype(np.float32)
        return (vv * (np.asarray(s0, np.float32)
                      + np.asarray(s1, np.float32) * vv)).astype(np.float32)

    def _mk(name, spec):
        shas = {}
        for ver in ("v3", "v4"):
            probe = DveOpSpec(name=name, opcode=0,
                              uops=lower(spec, ver=ver), rd1_en=has_src1(spec))
            shas[ver] = probe.sha(ver)
        op = dve_ops.DveOp(name, spec, subdim=False, uops_sha=shas)
        dve_ops.OPS.append(op)
        dve_ops.CUSTOM_DVE_SPECS[name] = spec
        row = dve_ops._CUSTOM_DVE_ROW_BASE + len(dve_ops.OPS) - 1
        assert row < 0x20
        dve_ops._SUB_OPCODE_FOR_NAME[name] = row
        return op

    # single-source C op (chain head, no accumulate): out = C0*min(t+|t|,C1)^3
    t3 = Src0 - C2
    v3 = minn(t3 + Bin(AluOp.ABSOLUTE_VALUE, t3, t3), C1)
    body_cs = (sq(v3) * v3) * C0

    def ref_cs(in0, in1, s0, s1, imm2):
        tt = in0.astype(np.float32) - np.float32(imm2)
        vv = np.minimum(tt + np.abs(tt), np.asarray(s1, np.float32))
        vv = vv.astype(np.float32)
        return (vv * vv * vv * np.asarray(s0, np.float32)).astype(np.float32)

    cb = _mk("BSPL_CB_ANT", Spec(body=body_cb, reference=ref_cb))
    ab = _mk("BSPL_ABS3_ANT",
             Spec(body=_spill_c3_to_src1(body_ab), reference=ref_ab))
    cs = _mk("BSPL_CS_ANT", Spec(body=body_cs, reference=ref_cs))
    return ab, cb, cs


# ---------------------------------------------------------------------------
# Bass kernel
# ---------------------------------------------------------------------------

_CACHE = {}


def _build_module(body_reps=1, nterms=None):
    key = ("e", body_reps, nterms)
    if key in _CACHE:
        return _CACHE[key]
    op_ab, op_cb, op_cs = _register_ops()
    from concourse import masks
    import contextlib

    terms = TERMS if nterms is None else TERMS[:nterms]
    # Piece-aligned accumulator chains: acc p owns piece p's AB head (the
    # single-source chain head) plus its C terms.  Emission alternates
    # between the two lowest-index still-open accs so acc 0 closes after
    # ~2x its term count; each acc's PE merge is emitted right after its
    # chain closes and overlaps the remaining DVE stream — only the last
    # acc's merge is tail-latency.
    import bisect
    ab_terms = [t for t in terms if t["kind"] == "ab"]
    assert len(ab_terms) == NACC
    kx_pieces = [float(_knots64()[2 * DG + s]) for s in PIECES]
    acc_terms = [[] for _ in range(NACC)]
    for t in terms:
        p = bisect.bisect_right(kx_pieces, t["imm2"] + 1e-12) - 1
        acc_terms[p].append(t)
    for p in range(NACC):
        # AB head must come first within its acc
        acc_terms[p].sort(key=lambda t: 0 if t["kind"] == "ab" else 1)
        assert acc_terms[p][0]["kind"] == "ab"

    def _emit_order():
        """Acc 4 (the PSUM-resident chain) takes every 5th slot so it closes
        on the final op; accs 0,1 alternate through the first half, 2,3
        through the second.  All same-acc spacings are >= 2 ops, so chained
        Src1 reads never stall on the previous write's ack."""
        n = sum(len(a) for a in acc_terms)
        rem = [len(a) for a in acc_terms]
        order = []
        phase = [0, 1]
        for pos in range(n):
            if pos % 5 == 4 and rem[4] > 0:
                p = 4
            else:
                act = [q for q in phase if rem[q] > 0]
                if not act:
                    phase = [2, 3]
                    act = [q for q in phase if rem[q] > 0]
                if not act:
                    act = [q for q in range(NACC) if rem[q] > 0]
                p = act[0]
                if order and order[-1] == p and len(act) > 1:
                    p = act[1]
            order.append(p)
            rem[p] -= 1
        return order

    nc = bacc.Bacc("TRN2", target_bir_lowering=False, debug=False,
                   num_devices=NCORES)
    x_in = nc.dram_tensor("x", [FSH, BSH], F32, kind="ExternalInput").ap()
    g_in = nc.dram_tensor("gtab", [FSH, NGCOL], F32, kind="ExternalInput").ap()
    y_out = nc.dram_tensor("y", [FSH, BSH], F32, kind="ExternalOutput").ap()

    with tile.TileContext(nc) as tc:
        with contextlib.ExitStack() as _st:
            const_pool = _st.enter_context(tc.tile_pool(name="const", bufs=1))
            x_pool = _st.enter_context(tc.tile_pool(name="x", bufs=2))
            z_pool = _st.enter_context(tc.tile_pool(name="z", bufs=2))
            a_pool = _st.enter_context(tc.tile_pool(name="a", bufs=2))
            out_pool = _st.enter_context(tc.tile_pool(name="out", bufs=2))
            ps_pool = _st.enter_context(
                tc.tile_pool(name="ps", bufs=2, space="PSUM"))

            identf = const_pool.tile([128, 128], F32)
            masks.make_identity(nc, identf[:])
            gtab = const_pool.tile([FSH, NGCOL], F32)
            # gtab on the ACT DMA ring so it loads in parallel with xt (SP ring)
            nc.scalar.dma_start(gtab[:], g_in[:])

            if body_reps > 1:
                _st.enter_context(tc.For_i(0, body_reps, 1))

            xt = x_pool.tile([FSH, BSH], F32, name="xt", tag="xt")
            nc.sync.dma_start(xt[:], x_in[:])

            # accs 0..3 get PE-merged into acc_ps as their chains close; acc 4
            # closes on the final DVE op and is read directly by the evac STT
            # (PSUM allows only one non-scalar STT input), so it needs no
            # merge at all.
            accs = [a_pool.tile([FSH, BSH], F32, name=f"acc{i}", tag=f"acc{i}")
                    for i in range(NACC)]
            acc_ps = ps_pool.tile([FSH, BSH], F32, name="accps", tag="accps")
            acc_e = accs[NACC - 1]

            ab_idx = {id(t): j for j, t in enumerate(ab_terms)}
            emitted = [0] * NACC
            merged = 0
            for p in _emit_order():
                tm = acc_terms[p][emitted[p]]
                acc = accs[p]
                if tm["kind"] == "ab":     # chain head: single-source, C3 bound
                    nab = ab_idx[id(tm)]
                    nc.vector._custom_dve(
                        op_ab, out=acc[:], in0=xt[:],
                        in1=gtab[:, NW2 + nab:NW2 + nab + 1],
                        s0=gtab[:, tm["a_col"]:tm["a_col"] + 1],
                        s1=gtab[:, tm["b_col"]:tm["b_col"] + 1],
                        imm2=tm["imm2"])
                else:
                    nc.vector._custom_dve(
                        op_cb, out=acc[:], in0=xt[:], in1=acc[:],
                        s0=gtab[:, tm["col"]:tm["col"] + 1],
                        s1=tm["s1"], imm2=tm["imm2"])
                emitted[p] += 1
                if emitted[p] == len(acc_terms[p]) and p < NACC - 1:
                    # SBUF chain p just closed: merge it into PSUM now so the
                    # matmuls overlap the remaining DVE stream
                    for h in range(2):
                        sl = slice(h * 512, (h + 1) * 512)
                        nc.tensor.matmul(acc_ps[:, sl], identf[:],
                                         acc[:, sl],
                                         start=(merged == 0),
                                         stop=(merged == NACC - 2))
                    merged += 1
            # evac on DVE: yt = (acc_ps + bias) + acc_e; replaces the ACT
            # identity evac and the 5th chain's merge
            yt = out_pool.tile([FSH, BSH], F32, name="yt", tag="yt")
            nc.vector.scalar_tensor_tensor(
                yt[:], acc_ps[:], gtab[:, CONST_COL:CONST_COL + 1], acc_e[:],
                op0=mybir.AluOpType.add, op1=mybir.AluOpType.add)
            nc.scalar.dma_start(y_out[:], yt[:])

    nc.compile()
    _CACHE[key] = nc
    return nc


# ---------------------------------------------------------------------------
# Public entry point
# ---------------------------------------------------------------------------

_CHECKED = False


def _make_in_maps(x, control_p, bias):
    global _CHECKED
    if not _CHECKED:
        _selfcheck()
        _CHECKED = True
    x = np.ascontiguousarray(x, dtype=np.float32)
    control_p = np.ascontiguousarray(control_p, dtype=np.float32)
    bias = np.ascontiguousarray(bias, dtype=np.float32)
    assert x.shape == (BATCH, NF) and control_p.shape == (NK, NF)
    w2 = _make_w2()     # (NK+1, NW2) float64
    in_maps, slots = [], []
    for c in range(NCORES):
        fh, bq = c // 4, c % 4
        fsl = slice(fh * FSH, (fh + 1) * FSH)
        bsl = slice(bq * BSH, (bq + 1) * BSH)
        cpb = np.concatenate([control_p[:, fsl], bias[None, fsl]],
                             axis=0).astype(np.float64)
        gtab = (cpb.T @ w2).astype(np.float32)
        bnds = np.zeros((FSH, len(AB_TERMS)), dtype=np.float32)
        for j, tm in enumerate(AB_TERMS):
            bnds[:, j] = np.float32(tm["bnd"])
        gtab = np.ascontiguousarray(np.concatenate([gtab, bnds], axis=1))
        in_maps.append({
            "x": np.ascontiguousarray(x[bsl, fsl].T),   # [FSH, BSH]
            "gtab": gtab,
        })
        slots.append((bsl, fsl))
    return in_maps, slots


def kernel(x, control_p, bias):
    nc = _build_module()
    in_maps, slots = _make_in_maps(x, control_p, bias)
    res = run_bass_kernel_spmd(nc, in_maps, list(range(NCORES)))

    out = np.empty((BATCH, NF), dtype=np.float32)
    for c, (bsl, fsl) in enumerate(slots):
        out[bsl, fsl] = res.results[c]["y"].T
    return out

